# revision 7
# baseline (speedup 1.0000x reference)
"""Trainium2 Bass kernel for a decoder block (MHA + FFN, causal, post-LN).

Sharding: 8 cores = 4 batches x 2 query-groups. Query 128-blocks are assigned
reflectively-striped so causal attention work is exactly balanced (68 k-blocks
per core) while every core runs the identical SPMD graph; the causal boundary
is handled by per-core mask *data* (tri / ones / zero tiles).

Each core computes K/V for the full sequence (per-head [64,64] projections are
cheap), attention for its own 1024 queries, then wo/LN1/FFN/LN2 token-locally.
No collectives. Host side only does layout prep: transposes, gathers, bf16
casts, bias reshapes.
"""

import numpy as np
from contextlib import ExitStack

import concourse.bass as bass
import concourse.bacc as bacc
import concourse.tile as tile
from concourse import mybir
from concourse.bass_utils import run_bass_kernel_spmd

import ml_dtypes

BF16 = ml_dtypes.bfloat16
P = 128


class Cfg:
    def __init__(self, B=4, S=2048, D=1024, H=16, DFF=4096):
        self.B, self.S, self.D, self.H, self.DFF = B, S, D, H, DFF
        self.HD = 64  # head dim (fixed: reference uses D//H = 64)
        assert D // H == self.HD
        self.NCORES = 2 * B
        self.SBT = S // P            # total q/k 128-blocks
        self.J = self.SBT // 2       # q-blocks per core
        self.J2 = self.J // 2
        self.CH = min(4, self.J)     # q-blocks per processing chunk (<=512 q)
        assert self.J % self.CH == 0
        self.NCH = self.J // self.CH
        self.DB = D // P             # d 128-blocks
        self.FB = DFF // P           # dff 128-blocks
        self.TQ = self.J * P         # tokens per core


def qsub_abs(cfg, j, parity):
    """absolute q 128-block index owned by (local j, core parity)."""
    if j < cfg.J2:
        return 2 * j + parity
    return 2 * j + 1 - parity


def chunks(n, c=512):
    return [(i, min(c, n - i)) for i in range(0, n, c)]


def build_nc(cfg: Cfg):
    nc = bacc.Bacc(None, target_bir_lowering=False, debug=True)
    f32, bf = mybir.dt.float32, mybir.dt.bfloat16
    S, D, H, HD, DFF = cfg.S, cfg.D, cfg.H, cfg.HD, cfg.DFF
    J, CH, NCH, DB, FB, SBT, TQ = cfg.J, cfg.CH, cfg.NCH, cfg.DB, cfg.FB, cfg.SBT, cfg.TQ

    # ---------------- DRAM parameters ----------------
    xT_d = nc.dram_tensor("xT", [D, S], bf, kind="ExternalInput")
    xTq_d = nc.dram_tensor("xTq", [D, TQ], bf, kind="ExternalInput")
    xres_d = nc.dram_tensor("xres", [TQ, D], f32, kind="ExternalInput")
    masks_d = nc.dram_tensor("masks", [J, 2, P, P], bf, kind="ExternalInput")
    wq_d = nc.dram_tensor("wq", [HD, HD], bf, kind="ExternalInput")
    wk_d = nc.dram_tensor("wk", [HD, HD], bf, kind="ExternalInput")
    wv_d = nc.dram_tensor("wv", [HD, HD], bf, kind="ExternalInput")
    bqkv_d = nc.dram_tensor("bqkv", [HD, 3], f32, kind="ExternalInput")  # cols: bq,bk,bv
    bv_row_d = nc.dram_tensor("bv_row", [1, HD], f32, kind="ExternalInput")
    wo_d = nc.dram_tensor("wo", [D, D], bf, kind="ExternalInput")
    w1_d = nc.dram_tensor("w1", [D, DFF], bf, kind="ExternalInput")
    b1_d = nc.dram_tensor("b1t", [P, FB], f32, kind="ExternalInput")
    w2_d = nc.dram_tensor("w2", [DFF, D], bf, kind="ExternalInput")
    # free-dim broadcast rows: bo, b2, ln1w, ln1b, ln2w, ln2b
    brow_d = nc.dram_tensor("brow", [6, D], f32, kind="ExternalInput")
    out_d = nc.dram_tensor("out", [TQ, D], f32, kind="ExternalOutput")

    def bcast(sl, parts=P):
        """DRAM AP broadcasting a [*] row slice across `parts` partitions."""
        return bass.AP(tensor=sl.tensor, offset=sl.offset, ap=[[0, parts]] + list(sl.ap))

    with tile.TileContext(nc) as tc, ExitStack() as ctx:
        # ------------- long-lived pools -------------
        consts = ctx.enter_context(tc.tile_pool(name="consts", bufs=1))
        persist = ctx.enter_context(tc.tile_pool(name="persist", bufs=1))

        bqkv = consts.tile([HD, 3], f32, tag="bqkv")
        nc.default_dma_engine.dma_start(out=bqkv, in_=bqkv_d[:])
        bv_bc = consts.tile([P, HD], f32, tag="bv_bc")
        nc.default_dma_engine.dma_start(out=bv_bc, in_=bcast(bv_row_d[0]))
        brows = []
        for i in range(6):
            t = consts.tile([P, D], f32, tag=f"brow{i}")
            nc.default_dma_engine.dma_start(out=t, in_=bcast(brow_d[i]))
            brows.append(t)
        bo_bc, b2_bc, ln1w, ln1b, ln2w, ln2b = brows
        b1t = consts.tile([P, FB], f32, tag="b1t")
        nc.default_dma_engine.dma_start(out=b1t, in_=b1_d[:])
        eps_t = consts.tile([P, 1], f32, tag="eps")
        nc.vector.memset(eps_t, 1e-5)
        wqkv_sb = {}
        for nm, dd in (("wq", wq_d), ("wk", wk_d), ("wv", wv_d)):
            t = consts.tile([HD, HD], bf, tag=nm)
            nc.default_dma_engine.dma_start(out=t, in_=dd[:])
            wqkv_sb[nm] = t

        # post-LN1 activations f32 (residual for FFN): live phase3..phase5
        x1f = [persist.tile([P, D], f32, tag=f"x1f{j}", name=f"x1f{j}") for j in range(J)]

        with tc.tile_pool(name="oallp", bufs=1) as oallp:
            # attention output, assembled q-major: J tiles [128 q, D] bf16
            o_all = [oallp.tile([P, D], bf, tag=f"oall{j}", name=f"oall{j}") for j in range(J)]

            # ================= phase 1: QKV + attention =================
            with tc.tile_pool(name="xtp", bufs=1) as xtp, \
                 tc.tile_pool(name="maskp", bufs=1) as maskp, \
                 tc.tile_pool(name="headp", bufs=2) as headp, \
                 tc.tile_pool(name="ptp", bufs=3) as ptp, \
                 tc.tile_pool(name="psA", bufs=2, space="PSUM") as psA, \
                 tc.tile_pool(name="psO", bufs=1, space="PSUM") as psO:

                mask_t = []
                for j in range(J):
                    pair = []
                    for m in range(2):
                        t = maskp.tile([P, P], bf, tag=f"mask{j}_{m}")
                        nc.default_dma_engine.dma_start(out=t, in_=masks_d[j, m])
                        pair.append(t)
                    mask_t.append(pair)

                xt = []
                for i in range(H):
                    t = xtp.tile([HD, S], bf, tag=f"xt{i}")
                    nc.default_dma_engine.dma_start(out=t, in_=xT_d[i * HD:(i + 1) * HD, :])
                    xt.append(t)
                xtq = []
                for i in range(H):
                    t = xtp.tile([HD, TQ], bf, tag=f"xtq{i}")
                    nc.default_dma_engine.dma_start(out=t, in_=xTq_d[i * HD:(i + 1) * HD, :])
                    xtq.append(t)

                for h in range(H):
                    xh_T = xt[h]    # [64, S] k/v source
                    xhq_T = xtq[h]  # [64, TQ] q source

                    # K^T [64, S]
                    kt = headp.tile([HD, S], bf, tag="kt")
                    for o_, w_ in chunks(S):
                        ps = psA.tile([P, 512], f32, tag="qkv")
                        nc.tensor.matmul(ps[:HD, :w_], wqkv_sb["wk"],
                                         xh_T[:, o_:o_ + w_],
                                         start=True, stop=True)
                        nc.vector.tensor_scalar(out=kt[:, o_:o_ + w_],
                                                in0=ps[:HD, :w_],
                                                scalar1=bqkv[:, 1:2], scalar2=None,
                                                op0=mybir.AluOpType.add)
                    # Q^T [64, TQ]
                    qt = headp.tile([HD, TQ], bf, tag="qt")
                    for o_, w_ in chunks(TQ):
                        ps = psA.tile([P, 512], f32, tag="qkv")
                        nc.tensor.matmul(ps[:HD, :w_], wqkv_sb["wq"],
                                         xhq_T[:, o_:o_ + w_],
                                         start=True, stop=True)
                        nc.vector.tensor_scalar(out=qt[:, o_:o_ + w_],
                                                in0=ps[:HD, :w_],
                                                scalar1=bqkv[:, 0:1], scalar2=None,
                                                op0=mybir.AluOpType.add)
                    # V1 [128, SBT, 65]: V rows + ones column
                    v1 = headp.tile([P, SBT, HD + 1], bf, tag="v1")
                    nc.vector.memset(v1[:, :, HD:HD + 1], 1.0)
                    for kb in range(SBT):
                        ps = psA.tile([P, 512], f32, tag="qkv")
                        nc.tensor.matmul(ps[:, :HD], xh_T[:, kb * P:(kb + 1) * P],
                                         wqkv_sb["wv"], start=True, stop=True)
                        nc.vector.tensor_add(out=v1[:, kb, :HD], in0=ps[:, :HD], in1=bv_bc)

                    # attention per chunk of CH q-blocks
                    for cc in range(NCH):
                        jhi = cc * CH + CH - 1
                        nkb = 2 * jhi + 2
                        o_ps = [psO.tile([P, HD + 1], f32, tag=f"o{jj}", name=f"ops{jj}")
                                for jj in range(CH)]
                        for kb in range(nkb):
                            jlo = max(cc * CH, kb // 2)
                            c0 = (jlo - cc * CH) * P   # first live column in chunk
                            ncols = (CH - (jlo - cc * CH)) * P
                            sc = psA.tile([P, 512], f32, tag="sc")
                            nc.tensor.matmul(
                                sc[:, c0:c0 + ncols], kt[:, kb * P:(kb + 1) * P],
                                qt[:, cc * CH * P + c0: cc * CH * P + c0 + ncols],
                                start=True, stop=True)
                            pt = ptp.tile([P, CH * P], bf, tag="pt")
                            nc.scalar.activation(out=pt[:, c0:c0 + ncols],
                                                 in_=sc[:, c0:c0 + ncols],
                                                 func=mybir.ActivationFunctionType.Exp,
                                                 scale=float(1.0 / np.sqrt(HD)))
                            jm = kb // 2
                            if cc * CH <= jm <= jhi:
                                mc0 = (jm - cc * CH) * P
                                nc.vector.tensor_mul(out=pt[:, mc0:mc0 + P],
                                                     in0=pt[:, mc0:mc0 + P],
                                                     in1=mask_t[jm][kb % 2])
                            for j in range(jlo, jhi + 1):
                                jc = j - cc * CH
                                nc.tensor.matmul(o_ps[jc], pt[:, jc * P:(jc + 1) * P],
                                                 v1[:, kb, :],
                                                 start=(kb == 0), stop=(kb == 2 * j + 1))
                        for j in range(cc * CH, jhi + 1):
                            jc = j - cc * CH
                            rs = ptp.tile([P, 1], f32, tag="rsum")
                            nc.vector.reciprocal(out=rs, in_=o_ps[jc][:, HD:HD + 1])
                            nc.vector.tensor_scalar_mul(
                                out=o_all[j][:, h * HD:(h + 1) * HD],
                                in0=o_ps[jc][:, :HD], scalar1=rs)

            # ============ phase 2+3: transpose o, wo, residual, LN1 ============
            with tc.tile_pool(name="otp", bufs=1) as otp, \
                 tc.tile_pool(name="wop", bufs=1) as wop, \
                 tc.tile_pool(name="ph3", bufs=3) as ph3, \
                 tc.tile_pool(name="psW", bufs=2, space="PSUM") as psW:
                wo_sb = []
                for i in range(DB):
                    t = wop.tile([P, D], bf, tag=f"wo{i}")
                    nc.default_dma_engine.dma_start(out=t, in_=wo_d[i * P:(i + 1) * P, :])
                    wo_sb.append(t)
                oT = [otp.tile([P, TQ], bf, tag=f"oT{i}", name=f"oT{i}") for i in range(DB)]
                for db in range(DB):
                    for j in range(J):
                        nc.sync.dma_start_transpose(
                            out=oT[db][:, j * P:(j + 1) * P],
                            in_=o_all[j][:, db * P:(db + 1) * P])
                for j in range(J):
                    xa = ph3.tile([P, D], f32, tag="xa")
                    xr = ph3.tile([P, D], f32, tag="xr")
                    nc.default_dma_engine.dma_start(out=xr, in_=xres_d[j * P:(j + 1) * P, :])
                    for o_, w_ in chunks(D):
                        ps = psW.tile([P, 512], f32, tag="wo")
                        for db in range(DB):
                            nc.tensor.matmul(ps[:, :w_], oT[db][:, j * P:(j + 1) * P],
                                             wo_sb[db][:, o_:o_ + w_],
                                             start=(db == 0), stop=(db == DB - 1))
                        hs = slice(o_, o_ + w_)
                        nc.vector.tensor_add(out=xa[:, hs], in0=ps[:, :w_], in1=bo_bc[:, hs])
                        nc.vector.tensor_add(out=xa[:, hs], in0=xa[:, hs], in1=xr[:, hs])
                    # LN1 -> x1f[j]
                    nchk = len(chunks(D))
                    stats = ph3.tile([P, nchk, 6], f32, tag="stats")
                    for ci, (o_, w_) in enumerate(chunks(D)):
                        nc.vector.bn_stats(out=stats[:, ci, :],
                                           in_=xa[:, o_:o_ + w_])
                    mv = ph3.tile([P, 2], f32, tag="mv")
                    nc.vector.bn_aggr(out=mv, in_=stats)
                    std = ph3.tile([P, 1], f32, tag="std")
                    nc.scalar.activation(out=std, in_=mv[:, 1:2],
                                         func=mybir.ActivationFunctionType.Sqrt,
                                         bias=eps_t)
                    rstd = ph3.tile([P, 1], f32, tag="rstd")
                    nc.vector.reciprocal(out=rstd, in_=std)
                    nc.vector.tensor_scalar(out=x1f[j], in0=xa, scalar1=mv[:, 0:1],
                                            scalar2=rstd, op0=mybir.AluOpType.subtract,
                                            op1=mybir.AluOpType.mult)
                    nc.vector.tensor_mul(out=x1f[j], in0=x1f[j], in1=ln1w)
                    nc.vector.tensor_add(out=x1f[j], in0=x1f[j], in1=ln1b)

        # ================= phase 4+5: transpose x1, FFN, LN2 =================
        with tc.tile_pool(name="x1tp", bufs=1) as x1tp, \
             tc.tile_pool(name="hgp", bufs=1) as hgp, \
             tc.tile_pool(name="w1p", bufs=3) as w1p, \
             tc.tile_pool(name="w2p", bufs=1) as w2p, \
             tc.tile_pool(name="ph5", bufs=3) as ph5, \
             tc.tile_pool(name="psH", bufs=2, space="PSUM") as psH, \
             tc.tile_pool(name="psY", bufs=4, space="PSUM") as psY:
            x1T = [x1tp.tile([P, TQ], bf, tag=f"x1T{i}", name=f"x1T{i}") for i in range(DB)]
            with tc.tile_pool(name="x1bp", bufs=1) as x1bp:
                x1b = [x1bp.tile([P, D], bf, tag=f"x1b{j}", name=f"x1b{j}") for j in range(J)]
                for j in range(J):
                    nc.scalar.activation(out=x1b[j], in_=x1f[j],
                                         func=mybir.ActivationFunctionType.Copy)
                for db in range(DB):
                    for j in range(J):
                        nc.sync.dma_start_transpose(
                            out=x1T[db][:, j * P:(j + 1) * P],
                            in_=x1b[j][:, db * P:(db + 1) * P])

            w2_sb = [w2p.tile([P, D], bf, tag=f"w2_{fb}", name=f"w2s{fb}") for fb in range(FB)]
            for fb in range(FB):
                nc.default_dma_engine.dma_start(out=w2_sb[fb],
                                                in_=w2_d[fb * P:(fb + 1) * P, :])
            for cc in range(NCH):
                qs = slice(cc * CH * P, (cc + 1) * CH * P)
                hg = [hgp.tile([P, CH * P], bf, tag=f"hg{fb}", name=f"hg{fb}") for fb in range(FB)]
                for fb in range(FB):
                    w1t = w1p.tile([P, DB, P], bf, tag="w1t")
                    nc.default_dma_engine.dma_start(
                        out=w1t,
                        in_=w1_d[:, fb * P:(fb + 1) * P].rearrange("(i p) c -> p i c", p=P))
                    ps = psH.tile([P, CH * P], f32, tag="h")
                    for db in range(DB):
                        nc.tensor.matmul(ps, w1t[:, db, :], x1T[db][:, qs],
                                         start=(db == 0), stop=(db == DB - 1))
                    nc.scalar.activation(out=hg[fb], in_=ps,
                                         func=mybir.ActivationFunctionType.Gelu,
                                         bias=b1t[:, fb:fb + 1])
                for jc in range(CH):
                    j = cc * CH + jc
                    for o_, w_ in chunks(D):
                        yps = psY.tile([P, 512], f32, tag="y")
                        for fb in range(FB):
                            nc.tensor.matmul(yps[:, :w_], hg[fb][:, jc * P:(jc + 1) * P],
                                             w2_sb[fb][:, o_:o_ + w_],
                                             start=(fb == 0), stop=(fb == FB - 1))
                        hs = slice(o_, o_ + w_)
                        nc.vector.tensor_add(out=x1f[j][:, hs], in0=yps[:, :w_],
                                             in1=x1f[j][:, hs])
                        nc.vector.tensor_add(out=x1f[j][:, hs], in0=x1f[j][:, hs],
                                             in1=b2_bc[:, hs])
                    # LN2 -> out
                    nchk = len(chunks(D))
                    stats = ph5.tile([P, nchk, 6], f32, tag="stats2")
                    for ci, (o_, w_) in enumerate(chunks(D)):
                        nc.vector.bn_stats(out=stats[:, ci, :],
                                           in_=x1f[j][:, o_:o_ + w_])
                    mv = ph5.tile([P, 2], f32, tag="mv2")
                    nc.vector.bn_aggr(out=mv, in_=stats)
                    std = ph5.tile([P, 1], f32, tag="std2")
                    nc.scalar.activation(out=std, in_=mv[:, 1:2],
                                         func=mybir.ActivationFunctionType.Sqrt,
                                         bias=eps_t)
                    rstd = ph5.tile([P, 1], f32, tag="rstd2")
                    nc.vector.reciprocal(out=rstd, in_=std)
                    of = ph5.tile([P, D], f32, tag="of")
                    nc.vector.tensor_scalar(out=of, in0=x1f[j], scalar1=mv[:, 0:1],
                                            scalar2=rstd, op0=mybir.AluOpType.subtract,
                                            op1=mybir.AluOpType.mult)
                    nc.vector.tensor_mul(out=of, in0=of, in1=ln2w)
                    nc.vector.tensor_add(out=of, in0=of, in1=ln2b)
                    nc.default_dma_engine.dma_start(out=out_d[j * P:(j + 1) * P, :], in_=of)
    nc.compile()
    return nc


# ---------------- host side ----------------

def make_masks(cfg, parity):
    # PT layout is [k, q]; keep where k <= q  -> upper-tri incl diagonal
    tri = np.triu(np.ones((P, P), np.float32))
    ones = np.ones((P, P), np.float32)
    zero = np.zeros((P, P), np.float32)
    out = np.empty((cfg.J, 2, P, P), np.float32)
    for j in range(cfg.J):
        a = qsub_abs(cfg, j, parity)
        if a == 2 * j:
            out[j, 0], out[j, 1] = tri, zero
        else:
            assert a == 2 * j + 1
            out[j, 0], out[j, 1] = ones, tri
    return out.astype(BF16)


def make_in_maps(cfg, inputs):
    g = lambda k: np.asarray(inputs[k], np.float32)
    x = g("x")
    brow = np.stack([g("bo"), g("b2"), g("ln1_w"), g("ln1_b"), g("ln2_w"), g("ln2_b")])
    shared = dict(
        wq=g("wq").astype(BF16), wk=g("wk").astype(BF16), wv=g("wv").astype(BF16),
        bqkv=np.stack([g("bq"), g("bk"), g("bv")], axis=1).copy(),
        bv_row=g("bv").reshape(1, -1).copy(),
        wo=g("wo").astype(BF16), w1=g("w1").astype(BF16), w2=g("w2").astype(BF16),
        b1t=np.ascontiguousarray(g("b1").reshape(cfg.FB, P).T), brow=brow,
    )
    in_maps = []
    for c in range(cfg.NCORES):
        b, par = c // 2, c % 2
        subs = [qsub_abs(cfg, j, par) for j in range(cfg.J)]
        rows = np.concatenate([np.arange(a * P, (a + 1) * P) for a in subs])
        xb = x[b]
        m = dict(shared)
        m["xT"] = np.ascontiguousarray(xb.T).astype(BF16)
        m["xTq"] = np.ascontiguousarray(xb.T[:, rows]).astype(BF16)
        m["xres"] = np.ascontiguousarray(xb[rows])
        m["masks"] = make_masks(cfg, par)
        in_maps.append(m)
    return in_maps


def assemble_out(cfg, results):
    out = np.empty((cfg.B, cfg.S, cfg.D), np.float32)
    for c in range(cfg.NCORES):
        b, par = c // 2, c % 2
        oc = results[c]["out"]
        for j in range(cfg.J):
            a = qsub_abs(cfg, j, par)
            out[b, a * P:(a + 1) * P, :] = oc[j * P:(j + 1) * P, :]
    return out


_CACHE = {}


def kernel(**inputs):
    cfg = Cfg()
    if "nc" not in _CACHE:
        _CACHE["nc"] = build_nc(cfg)
    nc = _CACHE["nc"]
    in_maps = make_in_maps(cfg, inputs)
    res = run_bass_kernel_spmd(nc, in_maps, core_ids=list(range(cfg.NCORES)))
    return assemble_out(cfg, res.results)


# revision 10
# speedup vs baseline: 181.0835x; 181.0835x over previous
"""Trainium2 Bass kernel for a decoder block (MHA + FFN, causal, post-LN).

Sharding: 8 cores = 4 batches x 2 query-groups. Query 128-blocks are assigned
reflectively-striped so causal attention work is exactly balanced (68 k-blocks
per core) while every core runs the identical SPMD graph; the causal boundary
is handled by per-core mask *data* (tri / ones / zero tiles).

Each core computes K/V for the full sequence (per-head [64,64] projections are
cheap), attention for its own 1024 queries, then wo/LN1/FFN/LN2 token-locally.
No collectives. Host side only does layout prep: transposes, gathers, bf16
casts, bias reshapes.
"""

import numpy as np
from contextlib import ExitStack

import concourse.bass as bass
import concourse.bacc as bacc
import concourse.tile as tile
from concourse import mybir
from concourse.bass_utils import run_bass_kernel_spmd

import ml_dtypes

BF16 = ml_dtypes.bfloat16
P = 128
PACK_SC = True  # row-packed 2-head scores matmuls


class Cfg:
    def __init__(self, B=4, S=2048, D=1024, H=16, DFF=4096):
        self.B, self.S, self.D, self.H, self.DFF = B, S, D, H, DFF
        self.HD = 64  # head dim (fixed: reference uses D//H = 64)
        assert D // H == self.HD
        self.NCORES = 2 * B
        self.SBT = S // P            # total q/k 128-blocks
        self.J = self.SBT // 2       # q-blocks per core
        self.J2 = self.J // 2
        self.CH = min(4, self.J)     # q-blocks per processing chunk (<=512 q)
        assert self.J % self.CH == 0
        self.NCH = self.J // self.CH
        self.DB = D // P             # d 128-blocks
        self.FB = DFF // P           # dff 128-blocks
        self.TQ = self.J * P         # tokens per core


def qsub_abs(cfg, j, parity):
    """absolute q 128-block index owned by (local j, core parity)."""
    if j < cfg.J2:
        return 2 * j + parity
    return 2 * j + 1 - parity


def chunks(n, c=512):
    return [(i, min(c, n - i)) for i in range(0, n, c)]


def build_nc(cfg: Cfg):
    nc = bacc.Bacc(None, target_bir_lowering=False, debug=True)
    f32, bf = mybir.dt.float32, mybir.dt.bfloat16
    S, D, H, HD, DFF = cfg.S, cfg.D, cfg.H, cfg.HD, cfg.DFF
    J, CH, NCH, DB, FB, SBT, TQ = cfg.J, cfg.CH, cfg.NCH, cfg.DB, cfg.FB, cfg.SBT, cfg.TQ

    # ---------------- DRAM parameters ----------------
    xT_d = nc.dram_tensor("xT", [D, S], bf, kind="ExternalInput")
    xTq_d = nc.dram_tensor("xTq", [D, TQ], bf, kind="ExternalInput")
    xres_d = nc.dram_tensor("xres", [TQ, D], f32, kind="ExternalInput")
    masks_d = nc.dram_tensor("masks", [J, 2, P, P], bf, kind="ExternalInput")
    wq_d = nc.dram_tensor("wq", [HD, HD], bf, kind="ExternalInput")
    wk_d = nc.dram_tensor("wk", [HD, HD], bf, kind="ExternalInput")
    wv_d = nc.dram_tensor("wv", [HD, HD], bf, kind="ExternalInput")
    bqkv_d = nc.dram_tensor("bqkv", [HD, 3], f32, kind="ExternalInput")  # cols: bq,bk,bv
    bv_row_d = nc.dram_tensor("bv_row", [1, HD], f32, kind="ExternalInput")
    wo_d = nc.dram_tensor("wo", [D, D], bf, kind="ExternalInput")
    w1_d = nc.dram_tensor("w1", [D, DFF], bf, kind="ExternalInput")
    b1_d = nc.dram_tensor("b1t", [P, FB], f32, kind="ExternalInput")
    w2_d = nc.dram_tensor("w2", [DFF, D], bf, kind="ExternalInput")
    # free-dim broadcast rows: bo, b2, ln1w, ln1b, ln2w, ln2b
    brow_d = nc.dram_tensor("brow", [6, D], f32, kind="ExternalInput")
    out_d = nc.dram_tensor("out", [TQ, D], f32, kind="ExternalOutput")

    def bcast(sl, parts=P):
        """DRAM AP broadcasting a [*] row slice across `parts` partitions."""
        return bass.AP(tensor=sl.tensor, offset=sl.offset, ap=[[0, parts]] + list(sl.ap))

    with tile.TileContext(nc) as tc, ExitStack() as ctx:
        # ------------- long-lived pools -------------
        consts = ctx.enter_context(tc.tile_pool(name="consts", bufs=1))
        persist = ctx.enter_context(tc.tile_pool(name="persist", bufs=1))

        bqkv = consts.tile([HD, 3], f32, tag="bqkv")
        nc.default_dma_engine.dma_start(out=bqkv, in_=bqkv_d[:])
        bv_bc = consts.tile([P, HD], f32, tag="bv_bc")
        nc.default_dma_engine.dma_start(out=bv_bc, in_=bcast(bv_row_d[0]))
        brows = []
        for i in range(6):
            t = consts.tile([P, D], f32, tag=f"brow{i}")
            nc.default_dma_engine.dma_start(out=t, in_=bcast(brow_d[i]))
            brows.append(t)
        bo_bc, b2_bc, ln1w, ln1b, ln2w, ln2b = brows
        b1t = consts.tile([P, FB], f32, tag="b1t")
        nc.default_dma_engine.dma_start(out=b1t, in_=b1_d[:])
        eps_t = consts.tile([P, 1], f32, tag="eps")
        nc.vector.memset(eps_t, 1e-5)
        wqkv_sb = {}
        for nm, dd in (("wq", wq_d), ("wk", wk_d), ("wv", wv_d)):
            t = consts.tile([HD, HD], bf, tag=nm)
            nc.default_dma_engine.dma_start(out=t, in_=dd[:])
            wqkv_sb[nm] = t

        # post-LN1 activations f32 (residual for FFN): live phase3..phase5
        x1f = [persist.tile([P, D], f32, tag=f"x1f{j}", name=f"x1f{j}") for j in range(J)]

        with tc.tile_pool(name="oallp", bufs=1) as oallp:
            # attention output, assembled q-major: J tiles [128 q, D] bf16
            o_all = [oallp.tile([P, D], bf, tag=f"oall{j}", name=f"oall{j}") for j in range(J)]

            # ================= phase 1: QKV + attention =================
            with tc.tile_pool(name="xtp", bufs=1) as xtp, \
                 tc.tile_pool(name="maskp", bufs=1) as maskp, \
                 tc.tile_pool(name="headp", bufs=2) as headp, \
                 tc.tile_pool(name="ptp", bufs=3) as ptp, \
                 tc.tile_pool(name="psA", bufs=2, space="PSUM") as psA, \
                 tc.tile_pool(name="psO", bufs=1, space="PSUM") as psO:

                mask_t = []
                for j in range(J):
                    pair = []
                    for m in range(2):
                        t = maskp.tile([P, P], bf, tag=f"mask{j}_{m}")
                        nc.default_dma_engine.dma_start(out=t, in_=masks_d[j, m])
                        pair.append(t)
                    mask_t.append(pair)

                xt = []
                for i in range(H):
                    t = xtp.tile([HD, S], bf, tag=f"xt{i}")
                    nc.default_dma_engine.dma_start(out=t, in_=xT_d[i * HD:(i + 1) * HD, :])
                    xt.append(t)
                xtq = []
                for i in range(H):
                    t = xtp.tile([HD, TQ], bf, tag=f"xtq{i}")
                    nc.default_dma_engine.dma_start(out=t, in_=xTq_d[i * HD:(i + 1) * HD, :])
                    xtq.append(t)

                for hp in range(H // 2):
                    # K^T / Q^T pair tiles: rows 0-63 head 2hp, rows 64-127 head 2hp+1
                    kt = headp.tile([P, S], bf, tag="kt")
                    qt = headp.tile([P, TQ], bf, tag="qt")
                    v1s = []
                    for hi in range(2):
                        h = 2 * hp + hi
                        xh_T, xhq_T = xt[h], xtq[h]
                        for o_, w_ in chunks(S):
                            ps = psA.tile([P, 512], f32, tag="qkv")
                            nc.tensor.matmul(ps[:HD, :w_], wqkv_sb["wk"],
                                             xh_T[:, o_:o_ + w_], start=True, stop=True)
                            if hi == 0:
                                nc.vector.tensor_scalar(
                                    out=kt[:HD, o_:o_ + w_], in0=ps[:HD, :w_],
                                    scalar1=bqkv[:, 1:2], scalar2=None,
                                    op0=mybir.AluOpType.add)
                            else:
                                kst = ptp.tile([HD, 512], bf, tag="kst")
                                nc.vector.tensor_scalar(
                                    out=kst[:, :w_], in0=ps[:HD, :w_],
                                    scalar1=bqkv[:, 1:2], scalar2=None,
                                    op0=mybir.AluOpType.add)
                                nc.sync.dma_start(out=kt[HD:, o_:o_ + w_],
                                                  in_=kst[:, :w_])
                        for o_, w_ in chunks(TQ):
                            ps = psA.tile([P, 512], f32, tag="qkv")
                            nc.tensor.matmul(ps[:HD, :w_], wqkv_sb["wq"],
                                             xhq_T[:, o_:o_ + w_], start=True, stop=True)
                            if hi == 0:
                                nc.vector.tensor_scalar(
                                    out=qt[:HD, o_:o_ + w_], in0=ps[:HD, :w_],
                                    scalar1=bqkv[:, 0:1], scalar2=None,
                                    op0=mybir.AluOpType.add)
                            else:
                                qst = ptp.tile([HD, 512], bf, tag="qst")
                                nc.vector.tensor_scalar(
                                    out=qst[:, :w_], in0=ps[:HD, :w_],
                                    scalar1=bqkv[:, 0:1], scalar2=None,
                                    op0=mybir.AluOpType.add)
                                nc.sync.dma_start(out=qt[HD:, o_:o_ + w_],
                                                  in_=qst[:, :w_])
                        v1 = headp.tile([P, SBT, HD + 1], bf, tag=f"v1{hi}",
                                        name=f"v1{hi}")
                        nc.vector.memset(v1[:, :, HD:HD + 1], 1.0)
                        for kb in range(SBT):
                            ps = psA.tile([P, 512], f32, tag="qkv")
                            nc.tensor.matmul(ps[:, :HD], xh_T[:, kb * P:(kb + 1) * P],
                                             wqkv_sb["wv"], start=True, stop=True)
                            nc.vector.tensor_add(out=v1[:, kb, :HD], in0=ps[:, :HD],
                                                 in1=bv_bc)
                        v1s.append(v1)
                    v1a, v1b = v1s

                    # attention per chunk of CH q-blocks, both heads of the pair
                    for cc in range(NCH):
                        jhi = cc * CH + CH - 1
                        nkb = 2 * jhi + 2
                        o4a = psO.tile([P, CH, HD + 1], f32, tag="o4a")
                        o4b = psO.tile([P, CH, HD + 1], f32, tag="o4b")
                        for kb in range(nkb):
                            jlo = max(cc * CH, kb // 2)
                            c0 = (jlo - cc * CH) * P   # first live column in chunk
                            ncols = (CH - (jlo - cc * CH)) * P
                            qt_ap0 = qt[:HD, cc * CH * P + c0: cc * CH * P + c0 + ncols]
                            qt_ap1 = qt[HD:, cc * CH * P + c0: cc * CH * P + c0 + ncols]
                            sca = psA.tile([P, 512], f32, tag="sc")
                            scb = psA.tile([P, 512], f32, tag="sc2")
                            if PACK_SC:
                                nc.tensor.matmul(sca[:, c0:c0 + ncols],
                                                 kt[:HD, kb * P:(kb + 1) * P], qt_ap0,
                                                 start=True, stop=True,
                                                 tile_position=(0, 0))
                                nc.tensor.matmul(scb[:, c0:c0 + ncols],
                                                 kt[HD:, kb * P:(kb + 1) * P], qt_ap1,
                                                 start=True, stop=True,
                                                 tile_position=(HD, 0))
                            else:
                                nc.tensor.matmul(sca[:, c0:c0 + ncols],
                                                 kt[:HD, kb * P:(kb + 1) * P], qt_ap0,
                                                 start=True, stop=True)
                                nc.tensor.matmul(scb[:, c0:c0 + ncols],
                                                 kt[HD:, kb * P:(kb + 1) * P], qt_ap1,
                                                 start=True, stop=True)
                            pta = ptp.tile([P, CH * P], bf, tag="pta")
                            ptb = ptp.tile([P, CH * P], bf, tag="ptb")
                            nc.scalar.activation(out=pta[:, c0:c0 + ncols],
                                                 in_=sca[:, c0:c0 + ncols],
                                                 func=mybir.ActivationFunctionType.Exp,
                                                 scale=float(1.0 / np.sqrt(HD)))
                            nc.scalar.activation(out=ptb[:, c0:c0 + ncols],
                                                 in_=scb[:, c0:c0 + ncols],
                                                 func=mybir.ActivationFunctionType.Exp,
                                                 scale=float(1.0 / np.sqrt(HD)))
                            jm = kb // 2
                            if cc * CH <= jm <= jhi:
                                mc0 = (jm - cc * CH) * P
                                nc.vector.tensor_mul(out=pta[:, mc0:mc0 + P],
                                                     in0=pta[:, mc0:mc0 + P],
                                                     in1=mask_t[jm][kb % 2])
                                nc.vector.tensor_mul(out=ptb[:, mc0:mc0 + P],
                                                     in0=ptb[:, mc0:mc0 + P],
                                                     in1=mask_t[jm][kb % 2])
                            for j in range(jlo, jhi + 1):
                                jc = j - cc * CH
                                st = (kb == 0 and j == cc * CH)
                                sp = (kb == nkb - 1 and j == jhi)
                                nc.tensor.matmul(o4a[:, jc, :],
                                                 pta[:, jc * P:(jc + 1) * P],
                                                 v1a[:, kb, :], start=st, stop=sp)
                                nc.tensor.matmul(o4b[:, jc, :],
                                                 ptb[:, jc * P:(jc + 1) * P],
                                                 v1b[:, kb, :], start=st, stop=sp)
                        for j in range(cc * CH, jhi + 1):
                            jc = j - cc * CH
                            for hh, o4 in ((2 * hp, o4a), (2 * hp + 1, o4b)):
                                rs = ptp.tile([P, 1], f32, tag="rsum")
                                nc.vector.reciprocal(out=rs, in_=o4[:, jc, HD:HD + 1])
                                nc.vector.tensor_scalar_mul(
                                    out=o_all[j][:, hh * HD:(hh + 1) * HD],
                                    in0=o4[:, jc, :HD], scalar1=rs)

            # ============ phase 2+3: transpose o, wo, residual, LN1 ============
            with tc.tile_pool(name="otp", bufs=1) as otp, \
                 tc.tile_pool(name="wop", bufs=1) as wop, \
                 tc.tile_pool(name="ph3", bufs=3) as ph3, \
                 tc.tile_pool(name="psW", bufs=2, space="PSUM") as psW:
                wo_sb = []
                for i in range(DB):
                    t = wop.tile([P, D], bf, tag=f"wo{i}")
                    nc.default_dma_engine.dma_start(out=t, in_=wo_d[i * P:(i + 1) * P, :])
                    wo_sb.append(t)
                oT = [otp.tile([P, TQ], bf, tag=f"oT{i}", name=f"oT{i}") for i in range(DB)]
                for db in range(DB):
                    for j in range(J):
                        nc.sync.dma_start_transpose(
                            out=oT[db][:, j * P:(j + 1) * P],
                            in_=o_all[j][:, db * P:(db + 1) * P])
                for j in range(J):
                    xa = ph3.tile([P, D], f32, tag="xa")
                    xr = ph3.tile([P, D], f32, tag="xr")
                    nc.default_dma_engine.dma_start(out=xr, in_=xres_d[j * P:(j + 1) * P, :])
                    for o_, w_ in chunks(D):
                        ps = psW.tile([P, 512], f32, tag="wo")
                        for db in range(DB):
                            nc.tensor.matmul(ps[:, :w_], oT[db][:, j * P:(j + 1) * P],
                                             wo_sb[db][:, o_:o_ + w_],
                                             start=(db == 0), stop=(db == DB - 1))
                        hs = slice(o_, o_ + w_)
                        nc.vector.tensor_add(out=xa[:, hs], in0=ps[:, :w_], in1=bo_bc[:, hs])
                        nc.vector.tensor_add(out=xa[:, hs], in0=xa[:, hs], in1=xr[:, hs])
                    # LN1 -> x1f[j]
                    nchk = len(chunks(D))
                    stats = ph3.tile([P, nchk, 6], f32, tag="stats")
                    for ci, (o_, w_) in enumerate(chunks(D)):
                        nc.vector.bn_stats(out=stats[:, ci, :],
                                           in_=xa[:, o_:o_ + w_])
                    mv = ph3.tile([P, 2], f32, tag="mv")
                    nc.vector.bn_aggr(out=mv, in_=stats)
                    std = ph3.tile([P, 1], f32, tag="std")
                    nc.scalar.activation(out=std, in_=mv[:, 1:2],
                                         func=mybir.ActivationFunctionType.Sqrt,
                                         bias=eps_t)
                    rstd = ph3.tile([P, 1], f32, tag="rstd")
                    nc.vector.reciprocal(out=rstd, in_=std)
                    nc.vector.tensor_scalar(out=x1f[j], in0=xa, scalar1=mv[:, 0:1],
                                            scalar2=rstd, op0=mybir.AluOpType.subtract,
                                            op1=mybir.AluOpType.mult)
                    nc.vector.tensor_mul(out=x1f[j], in0=x1f[j], in1=ln1w)
                    nc.vector.tensor_add(out=x1f[j], in0=x1f[j], in1=ln1b)

        # ================= phase 4+5: transpose x1, FFN, LN2 =================
        with tc.tile_pool(name="x1tp", bufs=1) as x1tp, \
             tc.tile_pool(name="hgp", bufs=1) as hgp, \
             tc.tile_pool(name="w1p", bufs=3) as w1p, \
             tc.tile_pool(name="w2p", bufs=1) as w2p, \
             tc.tile_pool(name="ph5", bufs=3) as ph5, \
             tc.tile_pool(name="psH", bufs=2, space="PSUM") as psH, \
             tc.tile_pool(name="psY", bufs=4, space="PSUM") as psY:
            x1T = [x1tp.tile([P, TQ], bf, tag=f"x1T{i}", name=f"x1T{i}") for i in range(DB)]
            with tc.tile_pool(name="x1bp", bufs=1) as x1bp:
                x1b = [x1bp.tile([P, D], bf, tag=f"x1b{j}", name=f"x1b{j}") for j in range(J)]
                for j in range(J):
                    nc.scalar.activation(out=x1b[j], in_=x1f[j],
                                         func=mybir.ActivationFunctionType.Copy)
                for db in range(DB):
                    for j in range(J):
                        nc.sync.dma_start_transpose(
                            out=x1T[db][:, j * P:(j + 1) * P],
                            in_=x1b[j][:, db * P:(db + 1) * P])

            w2_sb = [w2p.tile([P, D], bf, tag=f"w2_{fb}", name=f"w2s{fb}") for fb in range(FB)]
            for fb in range(FB):
                nc.default_dma_engine.dma_start(out=w2_sb[fb],
                                                in_=w2_d[fb * P:(fb + 1) * P, :])
            for cc in range(NCH):
                qs = slice(cc * CH * P, (cc + 1) * CH * P)
                hg = [hgp.tile([P, CH * P], bf, tag=f"hg{fb}", name=f"hg{fb}") for fb in range(FB)]
                for fb in range(FB):
                    w1t = w1p.tile([P, DB, P], bf, tag="w1t")
                    nc.default_dma_engine.dma_start(
                        out=w1t,
                        in_=w1_d[:, fb * P:(fb + 1) * P].rearrange("(i p) c -> p i c", p=P))
                    ps = psH.tile([P, CH * P], f32, tag="h")
                    for db in range(DB):
                        nc.tensor.matmul(ps, w1t[:, db, :], x1T[db][:, qs],
                                         start=(db == 0), stop=(db == DB - 1))
                    nc.scalar.activation(out=hg[fb], in_=ps,
                                         func=mybir.ActivationFunctionType.Gelu,
                                         bias=b1t[:, fb:fb + 1])
                for jc in range(CH):
                    j = cc * CH + jc
                    for o_, w_ in chunks(D):
                        yps = psY.tile([P, 512], f32, tag="y")
                        for fb in range(FB):
                            nc.tensor.matmul(yps[:, :w_], hg[fb][:, jc * P:(jc + 1) * P],
                                             w2_sb[fb][:, o_:o_ + w_],
                                             start=(fb == 0), stop=(fb == FB - 1))
                        hs = slice(o_, o_ + w_)
                        nc.vector.tensor_add(out=x1f[j][:, hs], in0=yps[:, :w_],
                                             in1=x1f[j][:, hs])
                        nc.vector.tensor_add(out=x1f[j][:, hs], in0=x1f[j][:, hs],
                                             in1=b2_bc[:, hs])
                    # LN2 -> out
                    nchk = len(chunks(D))
                    stats = ph5.tile([P, nchk, 6], f32, tag="stats2")
                    for ci, (o_, w_) in enumerate(chunks(D)):
                        nc.vector.bn_stats(out=stats[:, ci, :],
                                           in_=x1f[j][:, o_:o_ + w_])
                    mv = ph5.tile([P, 2], f32, tag="mv2")
                    nc.vector.bn_aggr(out=mv, in_=stats)
                    std = ph5.tile([P, 1], f32, tag="std2")
                    nc.scalar.activation(out=std, in_=mv[:, 1:2],
                                         func=mybir.ActivationFunctionType.Sqrt,
                                         bias=eps_t)
                    rstd = ph5.tile([P, 1], f32, tag="rstd2")
                    nc.vector.reciprocal(out=rstd, in_=std)
                    of = ph5.tile([P, D], f32, tag="of")
                    nc.vector.tensor_scalar(out=of, in0=x1f[j], scalar1=mv[:, 0:1],
                                            scalar2=rstd, op0=mybir.AluOpType.subtract,
                                            op1=mybir.AluOpType.mult)
                    nc.vector.tensor_mul(out=of, in0=of, in1=ln2w)
                    nc.vector.tensor_add(out=of, in0=of, in1=ln2b)
                    nc.default_dma_engine.dma_start(out=out_d[j * P:(j + 1) * P, :], in_=of)
    nc.compile()
    return nc


# ---------------- host side ----------------

def make_masks(cfg, parity):
    # PT layout is [k, q]; keep where k <= q  -> upper-tri incl diagonal
    tri = np.triu(np.ones((P, P), np.float32))
    ones = np.ones((P, P), np.float32)
    zero = np.zeros((P, P), np.float32)
    out = np.empty((cfg.J, 2, P, P), np.float32)
    for j in range(cfg.J):
        a = qsub_abs(cfg, j, parity)
        if a == 2 * j:
            out[j, 0], out[j, 1] = tri, zero
        else:
            assert a == 2 * j + 1
            out[j, 0], out[j, 1] = ones, tri
    return out.astype(BF16)


def make_in_maps(cfg, inputs):
    g = lambda k: np.asarray(inputs[k], np.float32)
    x = g("x")
    brow = np.stack([g("bo"), g("b2"), g("ln1_w"), g("ln1_b"), g("ln2_w"), g("ln2_b")])
    shared = dict(
        wq=g("wq").astype(BF16), wk=g("wk").astype(BF16), wv=g("wv").astype(BF16),
        bqkv=np.stack([g("bq"), g("bk"), g("bv")], axis=1).copy(),
        bv_row=g("bv").reshape(1, -1).copy(),
        wo=g("wo").astype(BF16), w1=g("w1").astype(BF16), w2=g("w2").astype(BF16),
        b1t=np.ascontiguousarray(g("b1").reshape(cfg.FB, P).T), brow=brow,
    )
    in_maps = []
    for c in range(cfg.NCORES):
        b, par = c // 2, c % 2
        subs = [qsub_abs(cfg, j, par) for j in range(cfg.J)]
        rows = np.concatenate([np.arange(a * P, (a + 1) * P) for a in subs])
        xb = x[b]
        m = dict(shared)
        m["xT"] = np.ascontiguousarray(xb.T).astype(BF16)
        m["xTq"] = np.ascontiguousarray(xb.T[:, rows]).astype(BF16)
        m["xres"] = np.ascontiguousarray(xb[rows])
        m["masks"] = make_masks(cfg, par)
        in_maps.append(m)
    return in_maps


def assemble_out(cfg, results):
    out = np.empty((cfg.B, cfg.S, cfg.D), np.float32)
    for c in range(cfg.NCORES):
        b, par = c // 2, c % 2
        oc = results[c]["out"]
        for j in range(cfg.J):
            a = qsub_abs(cfg, j, par)
            out[b, a * P:(a + 1) * P, :] = oc[j * P:(j + 1) * P, :]
    return out


_CACHE = {}


def kernel(**inputs):
    cfg = Cfg()
    if "nc" not in _CACHE:
        _CACHE["nc"] = build_nc(cfg)
    nc = _CACHE["nc"]
    in_maps = make_in_maps(cfg, inputs)
    res = run_bass_kernel_spmd(nc, in_maps, core_ids=list(range(cfg.NCORES)))
    return assemble_out(cfg, res.results)


# revision 11
# speedup vs baseline: 186.0201x; 1.0273x over previous
"""Trainium2 Bass kernel for a decoder block (MHA + FFN, causal, post-LN).

Sharding: 8 cores = 4 batches x 2 query-groups. Query 128-blocks are assigned
reflectively-striped so causal attention work is exactly balanced (68 k-blocks
per core) while every core runs the identical SPMD graph; the causal boundary
is handled by per-core mask *data* (tri / ones / zero tiles).

Each core computes K/V for the full sequence (per-head [64,64] projections are
cheap), attention for its own 1024 queries, then wo/LN1/FFN/LN2 token-locally.
No collectives. Host side only does layout prep: transposes, gathers, bf16
casts, bias reshapes.
"""

import numpy as np
from contextlib import ExitStack

import concourse.bass as bass
import concourse.bacc as bacc
import concourse.tile as tile
from concourse import mybir
from concourse.bass_utils import run_bass_kernel_spmd

import ml_dtypes

BF16 = ml_dtypes.bfloat16
P = 128
PACK_SC = True  # row-packed 2-head scores matmuls


class Cfg:
    def __init__(self, B=4, S=2048, D=1024, H=16, DFF=4096):
        self.B, self.S, self.D, self.H, self.DFF = B, S, D, H, DFF
        self.HD = 64  # head dim (fixed: reference uses D//H = 64)
        assert D // H == self.HD
        self.NCORES = 2 * B
        self.SBT = S // P            # total q/k 128-blocks
        self.J = self.SBT // 2       # q-blocks per core
        self.J2 = self.J // 2
        self.CH = min(4, self.J)     # q-blocks per processing chunk (<=512 q)
        assert self.J % self.CH == 0
        self.NCH = self.J // self.CH
        self.DB = D // P             # d 128-blocks
        self.FB = DFF // P           # dff 128-blocks
        self.TQ = self.J * P         # tokens per core


def qsub_abs(cfg, j, parity):
    """absolute q 128-block index owned by (local j, core parity)."""
    if j < cfg.J2:
        return 2 * j + parity
    return 2 * j + 1 - parity


def chunks(n, c=512):
    return [(i, min(c, n - i)) for i in range(0, n, c)]


def build_nc(cfg: Cfg):
    nc = bacc.Bacc(None, target_bir_lowering=False, debug=True)
    f32, bf = mybir.dt.float32, mybir.dt.bfloat16
    S, D, H, HD, DFF = cfg.S, cfg.D, cfg.H, cfg.HD, cfg.DFF
    J, CH, NCH, DB, FB, SBT, TQ = cfg.J, cfg.CH, cfg.NCH, cfg.DB, cfg.FB, cfg.SBT, cfg.TQ

    # ---------------- DRAM parameters ----------------
    xT_d = nc.dram_tensor("xT", [D, S], bf, kind="ExternalInput")
    xTq_d = nc.dram_tensor("xTq", [D, TQ], bf, kind="ExternalInput")
    xres_d = nc.dram_tensor("xres", [TQ, D], f32, kind="ExternalInput")
    masks_d = nc.dram_tensor("masks", [J, 2, P, P], bf, kind="ExternalInput")
    wq_d = nc.dram_tensor("wq", [HD, HD], bf, kind="ExternalInput")
    wk_d = nc.dram_tensor("wk", [HD, HD], bf, kind="ExternalInput")
    wv_d = nc.dram_tensor("wv", [HD, HD], bf, kind="ExternalInput")
    bqkv_d = nc.dram_tensor("bqkv", [HD, 3], f32, kind="ExternalInput")  # cols: bq,bk,bv
    bv_row_d = nc.dram_tensor("bv_row", [1, HD], f32, kind="ExternalInput")
    wo_d = nc.dram_tensor("wo", [D, D], bf, kind="ExternalInput")
    w1_d = nc.dram_tensor("w1", [D, DFF], bf, kind="ExternalInput")
    b1_d = nc.dram_tensor("b1t", [P, FB], f32, kind="ExternalInput")
    w2_d = nc.dram_tensor("w2", [DFF, D], bf, kind="ExternalInput")
    # free-dim broadcast rows: bo, b2, ln1w, ln1b, ln2w, ln2b
    brow_d = nc.dram_tensor("brow", [6, D], f32, kind="ExternalInput")
    out_d = nc.dram_tensor("out", [TQ, D], f32, kind="ExternalOutput")

    def bcast(sl, parts=P):
        """DRAM AP broadcasting a [*] row slice across `parts` partitions."""
        return bass.AP(tensor=sl.tensor, offset=sl.offset, ap=[[0, parts]] + list(sl.ap))

    with tile.TileContext(nc) as tc, ExitStack() as ctx:
        # ------------- long-lived pools -------------
        consts = ctx.enter_context(tc.tile_pool(name="consts", bufs=1))
        persist = ctx.enter_context(tc.tile_pool(name="persist", bufs=1))

        bqkv = consts.tile([HD, 3], f32, tag="bqkv")
        nc.default_dma_engine.dma_start(out=bqkv, in_=bqkv_d[:])
        bv_bc = consts.tile([P, HD], f32, tag="bv_bc")
        nc.default_dma_engine.dma_start(out=bv_bc, in_=bcast(bv_row_d[0]))
        brows = []
        for i in range(6):
            t = consts.tile([P, D], f32, tag=f"brow{i}")
            nc.default_dma_engine.dma_start(out=t, in_=bcast(brow_d[i]))
            brows.append(t)
        bo_bc, b2_bc, ln1w, ln1b, ln2w, ln2b = brows
        b1t = consts.tile([P, FB], f32, tag="b1t")
        nc.default_dma_engine.dma_start(out=b1t, in_=b1_d[:])
        eps_t = consts.tile([P, 1], f32, tag="eps")
        nc.vector.memset(eps_t, 1e-5)
        wqkv_sb = {}
        for nm, dd in (("wq", wq_d), ("wk", wk_d), ("wv", wv_d)):
            t = consts.tile([2 * HD, HD], bf, tag=nm)
            nc.default_dma_engine.dma_start(out=t[:HD, :], in_=dd[:])
            nc.default_dma_engine.dma_start(out=t[HD:, :], in_=dd[:])
            wqkv_sb[nm] = t

        # post-LN1 activations f32 (residual for FFN): live phase3..phase5
        x1f = [persist.tile([P, D], f32, tag=f"x1f{j}", name=f"x1f{j}") for j in range(J)]

        with tc.tile_pool(name="oallp", bufs=1) as oallp:
            # attention output, assembled q-major: J tiles [128 q, D] bf16
            o_all = [oallp.tile([P, D], bf, tag=f"oall{j}", name=f"oall{j}") for j in range(J)]

            # ================= phase 1: QKV + attention =================
            with tc.tile_pool(name="xtp", bufs=1) as xtp, \
                 tc.tile_pool(name="maskp", bufs=1) as maskp, \
                 tc.tile_pool(name="headp", bufs=2) as headp, \
                 tc.tile_pool(name="ptp", bufs=3) as ptp, \
                 tc.tile_pool(name="psA", bufs=2, space="PSUM") as psA, \
                 tc.tile_pool(name="psO", bufs=1, space="PSUM") as psO:

                mask_t = []
                for j in range(J):
                    pair = []
                    for m in range(2):
                        t = maskp.tile([P, P], bf, tag=f"mask{j}_{m}")
                        nc.default_dma_engine.dma_start(out=t, in_=masks_d[j, m])
                        pair.append(t)
                    mask_t.append(pair)

                xt = []
                for i in range(H // 2):
                    t = xtp.tile([P, S], bf, tag=f"xt{i}")
                    nc.default_dma_engine.dma_start(out=t, in_=xT_d[i * P:(i + 1) * P, :])
                    xt.append(t)
                xtq = []
                for i in range(H // 2):
                    t = xtp.tile([P, TQ], bf, tag=f"xtq{i}")
                    nc.default_dma_engine.dma_start(out=t, in_=xTq_d[i * P:(i + 1) * P, :])
                    xtq.append(t)

                for hp in range(H // 2):
                    # K^T / Q^T pair tiles: rows 0-63 head 2hp, rows 64-127 head 2hp+1
                    xp_T = xt[hp]
                    xqp_T = xtq[hp]
                    kt = headp.tile([P, S], bf, tag="kt")
                    qt = headp.tile([P, TQ], bf, tag="qt")
                    kst = headp.tile([HD, S], bf, tag="kst")
                    qst = headp.tile([HD, TQ], bf, tag="qst")
                    for o_, w_ in chunks(S):
                        psa_ = psA.tile([P, 512], f32, tag="qkv")
                        psb_ = psA.tile([P, 512], f32, tag="qkv")
                        nc.tensor.matmul(psa_[:HD, :w_], wqkv_sb["wk"][:HD, :],
                                         xp_T[:HD, o_:o_ + w_], start=True, stop=True,
                                         tile_position=(0, 0))
                        nc.tensor.matmul(psb_[:HD, :w_], wqkv_sb["wk"][HD:, :],
                                         xp_T[HD:, o_:o_ + w_], start=True, stop=True,
                                         tile_position=(HD, 0))
                        nc.vector.tensor_scalar(
                            out=kt[:HD, o_:o_ + w_], in0=psa_[:HD, :w_],
                            scalar1=bqkv[:, 1:2], scalar2=None, op0=mybir.AluOpType.add)
                        nc.vector.tensor_scalar(
                            out=kst[:, o_:o_ + w_], in0=psb_[:HD, :w_],
                            scalar1=bqkv[:, 1:2], scalar2=None, op0=mybir.AluOpType.add)
                    nc.sync.dma_start(out=kt[HD:, :], in_=kst[:, :])
                    for o_, w_ in chunks(TQ):
                        psa_ = psA.tile([P, 512], f32, tag="qkv")
                        psb_ = psA.tile([P, 512], f32, tag="qkv")
                        nc.tensor.matmul(psa_[:HD, :w_], wqkv_sb["wq"][:HD, :],
                                         xqp_T[:HD, o_:o_ + w_], start=True, stop=True,
                                         tile_position=(0, 0))
                        nc.tensor.matmul(psb_[:HD, :w_], wqkv_sb["wq"][HD:, :],
                                         xqp_T[HD:, o_:o_ + w_], start=True, stop=True,
                                         tile_position=(HD, 0))
                        nc.vector.tensor_scalar(
                            out=qt[:HD, o_:o_ + w_], in0=psa_[:HD, :w_],
                            scalar1=bqkv[:, 0:1], scalar2=None, op0=mybir.AluOpType.add)
                        nc.vector.tensor_scalar(
                            out=qst[:, o_:o_ + w_], in0=psb_[:HD, :w_],
                            scalar1=bqkv[:, 0:1], scalar2=None, op0=mybir.AluOpType.add)
                    nc.sync.dma_start(out=qt[HD:, :], in_=qst[:, :])
                    v1a = headp.tile([P, SBT, HD + 1], bf, tag="v1a")
                    v1b = headp.tile([P, SBT, HD + 1], bf, tag="v1b")
                    nc.vector.memset(v1a[:, :, HD:HD + 1], 1.0)
                    nc.vector.memset(v1b[:, :, HD:HD + 1], 1.0)
                    for kb in range(SBT):
                        psa_ = psA.tile([P, 512], f32, tag="qkv")
                        psb_ = psA.tile([P, 512], f32, tag="qkv")
                        nc.tensor.matmul(psa_[:, :HD], xp_T[:HD, kb * P:(kb + 1) * P],
                                         wqkv_sb["wv"][:HD, :], start=True, stop=True,
                                         tile_position=(0, 0))
                        nc.tensor.matmul(psb_[:, :HD], xp_T[HD:, kb * P:(kb + 1) * P],
                                         wqkv_sb["wv"][HD:, :], start=True, stop=True,
                                         tile_position=(HD, 0))
                        nc.vector.tensor_add(out=v1a[:, kb, :HD], in0=psa_[:, :HD],
                                             in1=bv_bc)
                        nc.vector.tensor_add(out=v1b[:, kb, :HD], in0=psb_[:, :HD],
                                             in1=bv_bc)

                    # attention per chunk of CH q-blocks, both heads of the pair
                    for cc in range(NCH):
                        jhi = cc * CH + CH - 1
                        nkb = 2 * jhi + 2
                        o4a = psO.tile([P, CH, HD + 1], f32, tag="o4a")
                        o4b = psO.tile([P, CH, HD + 1], f32, tag="o4b")
                        for kb in range(nkb):
                            jlo = max(cc * CH, kb // 2)
                            c0 = (jlo - cc * CH) * P   # first live column in chunk
                            ncols = (CH - (jlo - cc * CH)) * P
                            qt_ap0 = qt[:HD, cc * CH * P + c0: cc * CH * P + c0 + ncols]
                            qt_ap1 = qt[HD:, cc * CH * P + c0: cc * CH * P + c0 + ncols]
                            sca = psA.tile([P, 512], f32, tag="sc")
                            scb = psA.tile([P, 512], f32, tag="sc2")
                            if PACK_SC:
                                nc.tensor.matmul(sca[:, c0:c0 + ncols],
                                                 kt[:HD, kb * P:(kb + 1) * P], qt_ap0,
                                                 start=True, stop=True,
                                                 tile_position=(0, 0))
                                nc.tensor.matmul(scb[:, c0:c0 + ncols],
                                                 kt[HD:, kb * P:(kb + 1) * P], qt_ap1,
                                                 start=True, stop=True,
                                                 tile_position=(HD, 0))
                            else:
                                nc.tensor.matmul(sca[:, c0:c0 + ncols],
                                                 kt[:HD, kb * P:(kb + 1) * P], qt_ap0,
                                                 start=True, stop=True)
                                nc.tensor.matmul(scb[:, c0:c0 + ncols],
                                                 kt[HD:, kb * P:(kb + 1) * P], qt_ap1,
                                                 start=True, stop=True)
                            pta = ptp.tile([P, CH * P], bf, tag="pta")
                            ptb = ptp.tile([P, CH * P], bf, tag="ptb")
                            nc.scalar.activation(out=pta[:, c0:c0 + ncols],
                                                 in_=sca[:, c0:c0 + ncols],
                                                 func=mybir.ActivationFunctionType.Exp,
                                                 scale=float(1.0 / np.sqrt(HD)))
                            nc.scalar.activation(out=ptb[:, c0:c0 + ncols],
                                                 in_=scb[:, c0:c0 + ncols],
                                                 func=mybir.ActivationFunctionType.Exp,
                                                 scale=float(1.0 / np.sqrt(HD)))
                            jm = kb // 2
                            if cc * CH <= jm <= jhi:
                                mc0 = (jm - cc * CH) * P
                                nc.vector.tensor_mul(out=pta[:, mc0:mc0 + P],
                                                     in0=pta[:, mc0:mc0 + P],
                                                     in1=mask_t[jm][kb % 2])
                                nc.vector.tensor_mul(out=ptb[:, mc0:mc0 + P],
                                                     in0=ptb[:, mc0:mc0 + P],
                                                     in1=mask_t[jm][kb % 2])
                            for j in range(jlo, jhi + 1):
                                jc = j - cc * CH
                                st = (kb == 0 and j == cc * CH)
                                sp = (kb == nkb - 1 and j == jhi)
                                nc.tensor.matmul(o4a[:, jc, :],
                                                 pta[:, jc * P:(jc + 1) * P],
                                                 v1a[:, kb, :], start=st, stop=sp)
                                nc.tensor.matmul(o4b[:, jc, :],
                                                 ptb[:, jc * P:(jc + 1) * P],
                                                 v1b[:, kb, :], start=st, stop=sp)
                        for j in range(cc * CH, jhi + 1):
                            jc = j - cc * CH
                            for hh, o4 in ((2 * hp, o4a), (2 * hp + 1, o4b)):
                                rs = ptp.tile([P, 1], f32, tag="rsum")
                                nc.vector.reciprocal(out=rs, in_=o4[:, jc, HD:HD + 1])
                                nc.scalar.activation(
                                    out=o_all[j][:, hh * HD:(hh + 1) * HD],
                                    in_=o4[:, jc, :HD],
                                    func=mybir.ActivationFunctionType.Identity,
                                    scale=rs)

            # ============ phase 2+3: transpose o, wo, residual, LN1 ============
            with tc.tile_pool(name="otp", bufs=1) as otp, \
                 tc.tile_pool(name="wop", bufs=1) as wop, \
                 tc.tile_pool(name="ph3", bufs=3) as ph3, \
                 tc.tile_pool(name="psW", bufs=2, space="PSUM") as psW:
                wo_sb = []
                for i in range(DB):
                    t = wop.tile([P, D], bf, tag=f"wo{i}")
                    nc.default_dma_engine.dma_start(out=t, in_=wo_d[i * P:(i + 1) * P, :])
                    wo_sb.append(t)
                oT_all = otp.tile([P, DB, TQ], bf, tag="oT_all")
                for j in range(J):
                    nc.sync.dma_start_transpose(
                        out=oT_all[:, :, j * P:(j + 1) * P], in_=o_all[j][:, :])
                oT = [oT_all[:, i, :] for i in range(DB)]
                for j in range(J):
                    xa = ph3.tile([P, D], f32, tag="xa")
                    xr = ph3.tile([P, D], f32, tag="xr")
                    nc.default_dma_engine.dma_start(out=xr, in_=xres_d[j * P:(j + 1) * P, :])
                    for o_, w_ in chunks(D):
                        ps = psW.tile([P, 512], f32, tag="wo")
                        for db in range(DB):
                            nc.tensor.matmul(ps[:, :w_], oT[db][:, j * P:(j + 1) * P],
                                             wo_sb[db][:, o_:o_ + w_],
                                             start=(db == 0), stop=(db == DB - 1))
                        hs = slice(o_, o_ + w_)
                        nc.vector.tensor_add(out=xa[:, hs], in0=ps[:, :w_], in1=bo_bc[:, hs])
                        nc.vector.tensor_add(out=xa[:, hs], in0=xa[:, hs], in1=xr[:, hs])
                    # LN1 -> x1f[j]
                    nchk = len(chunks(D))
                    stats = ph3.tile([P, nchk, 6], f32, tag="stats")
                    for ci, (o_, w_) in enumerate(chunks(D)):
                        nc.vector.bn_stats(out=stats[:, ci, :],
                                           in_=xa[:, o_:o_ + w_])
                    mv = ph3.tile([P, 2], f32, tag="mv")
                    nc.vector.bn_aggr(out=mv, in_=stats)
                    std = ph3.tile([P, 1], f32, tag="std")
                    nc.scalar.activation(out=std, in_=mv[:, 1:2],
                                         func=mybir.ActivationFunctionType.Sqrt,
                                         bias=eps_t)
                    rstd = ph3.tile([P, 1], f32, tag="rstd")
                    nc.vector.reciprocal(out=rstd, in_=std)
                    nc.vector.tensor_scalar(out=x1f[j], in0=xa, scalar1=mv[:, 0:1],
                                            scalar2=rstd, op0=mybir.AluOpType.subtract,
                                            op1=mybir.AluOpType.mult)
                    nc.vector.tensor_mul(out=x1f[j], in0=x1f[j], in1=ln1w)
                    nc.vector.tensor_add(out=x1f[j], in0=x1f[j], in1=ln1b)

        # ================= phase 4+5: transpose x1, FFN, LN2 =================
        with tc.tile_pool(name="x1tp", bufs=1) as x1tp, \
             tc.tile_pool(name="hgp", bufs=1) as hgp, \
             tc.tile_pool(name="w1p", bufs=3) as w1p, \
             tc.tile_pool(name="w2p", bufs=1) as w2p, \
             tc.tile_pool(name="ph5", bufs=3) as ph5, \
             tc.tile_pool(name="psH", bufs=2, space="PSUM") as psH, \
             tc.tile_pool(name="psY", bufs=4, space="PSUM") as psY:
            x1T_all = x1tp.tile([P, DB, TQ], bf, tag="x1T_all")
            with tc.tile_pool(name="x1bp", bufs=1) as x1bp:
                x1b = [x1bp.tile([P, D], bf, tag=f"x1b{j}", name=f"x1b{j}") for j in range(J)]
                for j in range(J):
                    nc.scalar.activation(out=x1b[j], in_=x1f[j],
                                         func=mybir.ActivationFunctionType.Copy)
                for j in range(J):
                    nc.sync.dma_start_transpose(
                        out=x1T_all[:, :, j * P:(j + 1) * P], in_=x1b[j][:, :])
            x1T = [x1T_all[:, i, :] for i in range(DB)]

            w2_sb = [w2p.tile([P, D], bf, tag=f"w2_{fb}", name=f"w2s{fb}") for fb in range(FB)]
            for fb in range(FB):
                nc.default_dma_engine.dma_start(out=w2_sb[fb],
                                                in_=w2_d[fb * P:(fb + 1) * P, :])
            for cc in range(NCH):
                qs = slice(cc * CH * P, (cc + 1) * CH * P)
                hg = [hgp.tile([P, CH * P], bf, tag=f"hg{fb}", name=f"hg{fb}") for fb in range(FB)]
                for fb in range(FB):
                    w1t = w1p.tile([P, DB, P], bf, tag="w1t")
                    nc.default_dma_engine.dma_start(
                        out=w1t,
                        in_=w1_d[:, fb * P:(fb + 1) * P].rearrange("(i p) c -> p i c", p=P))
                    ps = psH.tile([P, CH * P], f32, tag="h")
                    for db in range(DB):
                        nc.tensor.matmul(ps, w1t[:, db, :], x1T[db][:, qs],
                                         start=(db == 0), stop=(db == DB - 1))
                    nc.scalar.activation(out=hg[fb], in_=ps,
                                         func=mybir.ActivationFunctionType.Gelu,
                                         bias=b1t[:, fb:fb + 1])
                for jc in range(CH):
                    j = cc * CH + jc
                    for o_, w_ in chunks(D):
                        yps = psY.tile([P, 512], f32, tag="y")
                        for fb in range(FB):
                            nc.tensor.matmul(yps[:, :w_], hg[fb][:, jc * P:(jc + 1) * P],
                                             w2_sb[fb][:, o_:o_ + w_],
                                             start=(fb == 0), stop=(fb == FB - 1))
                        hs = slice(o_, o_ + w_)
                        nc.vector.tensor_add(out=x1f[j][:, hs], in0=yps[:, :w_],
                                             in1=x1f[j][:, hs])
                        nc.vector.tensor_add(out=x1f[j][:, hs], in0=x1f[j][:, hs],
                                             in1=b2_bc[:, hs])
                    # LN2 -> out
                    nchk = len(chunks(D))
                    stats = ph5.tile([P, nchk, 6], f32, tag="stats2")
                    for ci, (o_, w_) in enumerate(chunks(D)):
                        nc.vector.bn_stats(out=stats[:, ci, :],
                                           in_=x1f[j][:, o_:o_ + w_])
                    mv = ph5.tile([P, 2], f32, tag="mv2")
                    nc.vector.bn_aggr(out=mv, in_=stats)
                    std = ph5.tile([P, 1], f32, tag="std2")
                    nc.scalar.activation(out=std, in_=mv[:, 1:2],
                                         func=mybir.ActivationFunctionType.Sqrt,
                                         bias=eps_t)
                    rstd = ph5.tile([P, 1], f32, tag="rstd2")
                    nc.vector.reciprocal(out=rstd, in_=std)
                    of = ph5.tile([P, D], f32, tag="of")
                    nc.vector.tensor_scalar(out=of, in0=x1f[j], scalar1=mv[:, 0:1],
                                            scalar2=rstd, op0=mybir.AluOpType.subtract,
                                            op1=mybir.AluOpType.mult)
                    nc.vector.tensor_mul(out=of, in0=of, in1=ln2w)
                    nc.vector.tensor_add(out=of, in0=of, in1=ln2b)
                    nc.default_dma_engine.dma_start(out=out_d[j * P:(j + 1) * P, :], in_=of)
    nc.compile()
    return nc


# ---------------- host side ----------------

def make_masks(cfg, parity):
    # PT layout is [k, q]; keep where k <= q  -> upper-tri incl diagonal
    tri = np.triu(np.ones((P, P), np.float32))
    ones = np.ones((P, P), np.float32)
    zero = np.zeros((P, P), np.float32)
    out = np.empty((cfg.J, 2, P, P), np.float32)
    for j in range(cfg.J):
        a = qsub_abs(cfg, j, parity)
        if a == 2 * j:
            out[j, 0], out[j, 1] = tri, zero
        else:
            assert a == 2 * j + 1
            out[j, 0], out[j, 1] = ones, tri
    return out.astype(BF16)


def make_in_maps(cfg, inputs):
    g = lambda k: np.asarray(inputs[k], np.float32)
    x = g("x")
    brow = np.stack([g("bo"), g("b2"), g("ln1_w"), g("ln1_b"), g("ln2_w"), g("ln2_b")])
    shared = dict(
        wq=g("wq").astype(BF16), wk=g("wk").astype(BF16), wv=g("wv").astype(BF16),
        bqkv=np.stack([g("bq"), g("bk"), g("bv")], axis=1).copy(),
        bv_row=g("bv").reshape(1, -1).copy(),
        wo=g("wo").astype(BF16), w1=g("w1").astype(BF16), w2=g("w2").astype(BF16),
        b1t=np.ascontiguousarray(g("b1").reshape(cfg.FB, P).T), brow=brow,
    )
    in_maps = []
    for c in range(cfg.NCORES):
        b, par = c // 2, c % 2
        subs = [qsub_abs(cfg, j, par) for j in range(cfg.J)]
        rows = np.concatenate([np.arange(a * P, (a + 1) * P) for a in subs])
        xb = x[b]
        m = dict(shared)
        m["xT"] = np.ascontiguousarray(xb.T).astype(BF16)
        m["xTq"] = np.ascontiguousarray(xb.T[:, rows]).astype(BF16)
        m["xres"] = np.ascontiguousarray(xb[rows])
        m["masks"] = make_masks(cfg, par)
        in_maps.append(m)
    return in_maps


def assemble_out(cfg, results):
    out = np.empty((cfg.B, cfg.S, cfg.D), np.float32)
    for c in range(cfg.NCORES):
        b, par = c // 2, c % 2
        oc = results[c]["out"]
        for j in range(cfg.J):
            a = qsub_abs(cfg, j, par)
            out[b, a * P:(a + 1) * P, :] = oc[j * P:(j + 1) * P, :]
    return out


_CACHE = {}


def kernel(**inputs):
    cfg = Cfg()
    if "nc" not in _CACHE:
        _CACHE["nc"] = build_nc(cfg)
    nc = _CACHE["nc"]
    in_maps = make_in_maps(cfg, inputs)
    res = run_bass_kernel_spmd(nc, in_maps, core_ids=list(range(cfg.NCORES)))
    return assemble_out(cfg, res.results)
